# revision 1
# baseline (speedup 1.0000x reference)
"""BiMamba block Trainium2 kernel — 8 NeuronCores.

Sharding: core = (direction, batch, d_inner-half):  c = 4*dir + 2*b + half.
Each core runs the full sequence (L=1024) for one direction of one batch
element over half of d_inner (1024 channels), so the selective scan has no
cross-core sequential dependency.  Cross-core sums use three small
collectives: AllReduce over (half) pairs for the x_proj partials, an
AllReduce-pair + AllGather across directions for the out_proj partials
(the backward direction's contribution is time-reversed on-chip while
combining), and an AllReduce over the batch quad for the FFN partials
(d_ff is sharded 4-way across the quad).

Layout on chip is feature-major: activations live as [feature partitions,
time free].  The scan runs on the Vector engine's tensor_tensor_scan with
free dim ordered (state n major, time minor); dA = exp(A*delta) comes from
the Scalar engine with the A column as the per-partition activation scale.
"""

import numpy as np
import ml_dtypes

import concourse.bass as bass
import concourse.bacc as bacc
import concourse.mybir as mybir
import concourse.tile as tile
from contextlib import ExitStack

F32 = mybir.dt.float32
F16 = mybir.dt.float16
BF16 = mybir.dt.bfloat16
Alu = mybir.AluOpType
Act = mybir.ActivationFunctionType

P = 128
DM = 1024          # d_model
L = 1024           # sequence length
DIH = 1024         # d_inner half (per core)
NST = 16           # d_state
DTR = 64           # dt_rank
KT = DM // P       # 8  k-tiles over d_model
DT = DIH // P      # 8  d-tiles over d_inner-half
TSUB = 256         # scan sub-chunk length
NSUB = L // TSUB   # 4
DFQ = 1024         # d_ff quarter (per core)
EPS = 1e-5

_BF = ml_dtypes.bfloat16

PAIRS = [[0, 1], [2, 3], [4, 5], [6, 7]]
XDIR = [[0, 4], [1, 5], [2, 6], [3, 7]]
QUADS = [[0, 1, 4, 5], [2, 3, 6, 7]]


def _ln_stats(nc, pools, get_x, ones_col, ones_bf, rstd_sb, mean_sb):
    """Per-token mean/rstd over KT tiles of [128, L] f32 (features on
    partitions).  get_x(kt) returns the kt-th SBUF tile.

    Writes mean_sb [128, L] and rstd_sb [128, L] (broadcast to all
    partitions via gpsimd.partition_broadcast).
    """
    psum, scratch, statp = pools["psum"], pools["scratch"], pools["statp"]
    ps_m = [psum.tile([1, 512], F32, tag="mm", name=f"ps_m{_h}")
            for _h in range(2)]
    ps_s = [psum.tile([1, 512], F32, tag="mm", name=f"ps_s{_h}")
            for _h in range(2)]
    F32R = mybir.dt.float32r
    for kt in range(KT):
        xkt = get_x(kt)
        sq = scratch.tile([P, L], BF16, tag="big", name="sq")
        nc.scalar.square(sq[:], xkt)
        for h in range(2):
            sl = slice(h * 512, (h + 1) * 512)
            nc.tensor.matmul(ps_m[h][:], ones_col[:], xkt[:, sl],
                             start=(kt == 0), stop=(kt == KT - 1))
            nc.tensor.matmul(ps_s[h][:], ones_bf[:], sq[:, sl],
                             start=(kt == 0), stop=(kt == KT - 1))
    mean1 = statp.tile([1, L], F32, tag="stat", name="mean1")
    var1 = statp.tile([1, L], F32, tag="stat", name="var1")
    m2 = statp.tile([1, L], F32, tag="stat", name="m2")
    for h in range(2):
        sl = slice(h * 512, (h + 1) * 512)
        nc.vector.tensor_scalar_mul(mean1[:, sl], ps_m[h][:], 1.0 / DM)
        nc.vector.tensor_scalar_mul(var1[:, sl], ps_s[h][:], 1.0 / DM)
    nc.vector.tensor_tensor(m2[:], mean1[:], mean1[:], Alu.mult)
    nc.vector.tensor_tensor(var1[:], var1[:], m2[:], Alu.subtract)
    nc.vector.tensor_scalar_add(var1[:], var1[:], EPS)
    nc.scalar.sqrt(var1[:], var1[:])
    nc.vector.reciprocal(m2[:], var1[:])
    nc.gpsimd.partition_broadcast(mean_sb[:], mean1[:])
    nc.gpsimd.partition_broadcast(rstd_sb[:], m2[:])


def _ln_apply(nc, pools, get_x, mean_sb, rstd_sb, out_sb):
    """out = (x - mean) * rstd, per kt tile; out_sb bf16.  The LN gain and
    bias are folded into the following matmul's weights and evacuation bias
    on the host."""
    scratch = pools["scratch"]
    for kt in range(KT):
        xkt = get_x(kt)
        t1 = scratch.tile([P, L], F32, tag="big", name="t1")
        nc.vector.tensor_tensor(t1[:], xkt, mean_sb[:], Alu.subtract)
        nc.vector.tensor_tensor(out_sb[:, kt, :], t1[:], rstd_sb[:],
                                Alu.mult)


def build_program(sim=False, reps=1):
    nc = bacc.Bacc("TRN2", target_bir_lowering=False, debug=False,
                   num_devices=8)

    def collective(kind, op, groups, ins, outs):
        if sim:
            # timing-only stand-in: the collectives' DMA footprint without
            # cross-core semantics (TimelineSim is single-core)
            nc.sync.dma_start(outs[0][:, 0:ins[0].shape[1]]
                              if outs[0].shape != ins[0].shape else outs[0],
                              ins[0])
        else:
            nc.gpsimd.collective_compute(kind, op, replica_groups=groups,
                                         ins=[ins[0].opt()],
                                         outs=[outs[0].opt()])

    # ---- I/O ----
    xT_ln = nc.dram_tensor("xT_ln", [DM, L], F32, kind="ExternalInput")
    xT_res = nc.dram_tensor("xT_res", [DM, L], F32, kind="ExternalInput")
    win_b = nc.dram_tensor("win_b", [2 * DIH], F32, kind="ExternalInput")
    win_t = nc.dram_tensor("win_t", [DM, 2 * DIH], BF16, kind="ExternalInput")
    conv_w = nc.dram_tensor("conv_w", [DIH, 4], F32, kind="ExternalInput")
    conv_b = nc.dram_tensor("conv_b", [DIH], F32, kind="ExternalInput")
    a_mat = nc.dram_tensor("a_mat", [DIH, NST], F32, kind="ExternalInput")
    xpw_t = nc.dram_tensor("xpw_t", [DIH, 96], BF16, kind="ExternalInput")
    dtw_t = nc.dram_tensor("dtw_t", [DTR, DIH], BF16, kind="ExternalInput")
    dt_b = nc.dram_tensor("dt_b", [DIH], F32, kind="ExternalInput")
    d_par = nc.dram_tensor("d_par", [DIH], F32, kind="ExternalInput")
    outw_t = nc.dram_tensor("outw_t", [DIH, DM], BF16, kind="ExternalInput")
    w1_t = nc.dram_tensor("w1_t", [DM, DFQ], BF16, kind="ExternalInput")
    b1_q = nc.dram_tensor("b1_q", [DFQ], F32, kind="ExternalInput")
    w2_t = nc.dram_tensor("w2_t", [DFQ, DM], BF16, kind="ExternalInput")
    b2_e = nc.dram_tensor("b2_e", [DM], F32, kind="ExternalInput")
    outT = nc.dram_tensor("outT", [DM, L], F32, kind="ExternalOutput")

    def vec_pt(dram_vec, pool, dt_, tag):
        """Load a [D] dram vector as [128, D//128] (col o = chans o*128..)."""
        t = pool.tile([P, dram_vec.shape[0] // P], dt_, tag=tag, name=tag)
        nc.sync.dma_start(t[:], dram_vec.rearrange("(o p) -> p o", p=P))
        return t

    with tile.TileContext(nc) as tc, ExitStack() as es:
        pc = es.enter_context(tc.tile_pool(name="const", bufs=1))
        psum = es.enter_context(tc.tile_pool(name="psum", bufs=8, space="PSUM"))
        scratch = es.enter_context(tc.tile_pool(name="scratch", bufs=4))
        statp = es.enter_context(tc.tile_pool(name="statp", bufs=3))
        dram = es.enter_context(tc.tile_pool(name="dram", bufs=1, space="DRAM"))
        pools = {"psum": psum, "scratch": scratch, "statp": statp}
        for _rep in range(reps):
            _body(nc, tc, sim, collective, pools, pc, psum, scratch, statp,
                  dram, xT_ln, xT_res, win_b, win_t, conv_w, conv_b, a_mat,
                  xpw_t, dtw_t, dt_b, d_par, outw_t, w1_t, b1_q, w2_t, b2_e,
                  outT, vec_pt)

    nc.compile()
    return nc


def _body(nc, tc, sim, collective, pools, pc, psum, scratch, statp, dram,
          xT_ln, xT_res, win_b, win_t, conv_w, conv_b, a_mat, xpw_t, dtw_t,
          dt_b, d_par, outw_t, w1_t, b1_q, w2_t, b2_e, outT, vec_pt):
    if True:

        # constants
        ones_col = pc.tile([P, 1], F32, tag="ones")
        nc.vector.memset(ones_col[:], 1.0)
        ones_bf = pc.tile([P, 1], BF16, tag="onesb")
        nc.vector.memset(ones_bf[:], 1.0)
        wbv = vec_pt(win_b, pc, F32, "wbv")
        cw = pc.tile([P, DT, 4], F32, tag="cw")
        nc.sync.dma_start(cw[:], conv_w.rearrange("(o p) k -> p o k", p=P))
        cb = vec_pt(conv_b, pc, F32, "cb")
        a_sb = pc.tile([P, DT, NST], F32, tag="a")
        nc.sync.dma_start(a_sb[:], a_mat.rearrange("(o p) n -> p o n", p=P))
        dtb = vec_pt(dt_b, pc, F32, "dtb")
        dpv = vec_pt(d_par, pc, F32, "dpv")
        b1s = vec_pt(b1_q, pc, F32, "b1s")
        b2s = vec_pt(b2_e, pc, F32, "b2s")

        bc_dram = dram.tile([2 * NST, L], BF16)

        with tc.tile_pool(name="pD", bufs=1) as pD:
            y_g = pD.tile([P, DT, L], BF16, tag="yg")
            outw = pD.tile([P, DT, DM], BF16, tag="outw")

            with tc.tile_pool(name="pact", bufs=1) as pact:
                # persistent activations for the mamba branch
                delta = pact.tile([P, DT, L], F16, tag="delta")
                u_bf = pact.tile([P, DT, L], BF16, tag="u")
                silz = pact.tile([P, DT, L], BF16, tag="silz")

                # ---------- Phase 0: LN1 ----------
                with tc.tile_pool(name="pA", bufs=1) as pA, \
                     tc.tile_pool(name="pAx", bufs=3) as pAx:
                    xlnv = xT_ln.rearrange("(o p) t -> p o t", p=P)

                    def _load_x1(kt):
                        xk = pAx.tile([P, L], F32, tag="xk", name="xk")
                        nc.sync.dma_start(xk[:], xlnv[:, kt, :])
                        return xk[:]

                    mean_sb = pA.tile([P, L], F32, tag="meanr")
                    rstd_sb = pA.tile([P, L], F32, tag="rstdr")
                    _ln_stats(nc, pools, _load_x1, ones_col, ones_bf, rstd_sb, mean_sb)
                    xnorm = pA.tile([P, KT, L], BF16, tag="xnorm")
                    _ln_apply(nc, pools, _load_x1, mean_sb, rstd_sb, xnorm)

                    # ---------- Phase 1: in_proj ----------
                    with tc.tile_pool(name="pW1", bufs=1) as pW1, \
                         tc.tile_pool(name="pB", bufs=1) as pB:
                        win = pW1.tile([P, KT, 2 * DIH], BF16, tag="win")
                        nc.sync.dma_start(
                            win[:], win_t.rearrange("(o p) e -> p o e", p=P))
                        xiT = pB.tile([P, DT, L + 3], BF16, tag="xi")
                        nc.vector.memset(xiT[:, :, 0:3], 0.0)
                        for m in range(2 * DT):
                            pss = [psum.tile([P, 512], F32, tag="mm",
                                             name="pss%d" % _h)
                                   for _h in range(2)]
                            for kt in range(KT):
                                for h in range(2):
                                    nc.tensor.matmul(
                                        pss[h][:],
                                        win[:, kt, m * P:(m + 1) * P],
                                        xnorm[:, kt, h * 512:(h + 1) * 512],
                                        start=(kt == 0), stop=(kt == KT - 1))
                            for h in range(2):
                                if m < DT:
                                    nc.scalar.activation(
                                        xiT[:, m,
                                            3 + h * 512:3 + (h + 1) * 512],
                                        pss[h][:], Act.Identity,
                                        bias=wbv[:, m:m + 1])
                                else:
                                    nc.scalar.activation(
                                        silz[:, m - DT,
                                             h * 512:(h + 1) * 512],
                                        pss[h][:], Act.Silu,
                                        bias=wbv[:, m:m + 1])

                        # ---------- Phase 2: conv + silu ----------
                        for d in range(DT):
                            acc = scratch.tile([P, L], BF16, tag="big",
                                               name="acc")
                            nc.vector.tensor_scalar(
                                acc[:], xiT[:, d, 0:L], cw[:, d, 0:1],
                                cb[:, d:d + 1], Alu.mult, Alu.add)
                            for k in (1, 2, 3):
                                nc.vector.scalar_tensor_tensor(
                                    acc[:], xiT[:, d, k:k + L],
                                    cw[:, d, k:k + 1], acc[:],
                                    Alu.mult, Alu.add)
                            nc.scalar.activation(u_bf[:, d, :], acc[:],
                                                 Act.Silu)

                        # ------- Phase 3: x_proj (+ pair AllReduce) -------
                        xpw = pB.tile([P, DT, 96], BF16, tag="xpw")
                        nc.sync.dma_start(
                            xpw[:], xpw_t.rearrange("(o p) r -> p o r", p=P))
                        psx = [psum.tile([96, 512], F32, tag="mm",
                                         name="psx%d" % _h) for _h in range(2)]
                        for d in range(DT):
                            for h in range(2):
                                nc.tensor.matmul(
                                    psx[h][:], xpw[:, d, :],
                                    u_bf[:, d, h * 512:(h + 1) * 512],
                                    start=(d == 0), stop=(d == DT - 1))
                        xd_in = dram.tile([96, L], F32)
                        xd_out = dram.tile([96, L], F32)
                        xdp = scratch.tile([P, L], F32, tag="big", name="xdp")
                        for h in range(2):
                            nc.scalar.copy(
                                xdp[0:96, h * 512:(h + 1) * 512], psx[h][:])
                        nc.sync.dma_start(xd_in[:], xdp[0:96, :])
                        collective("AllReduce", Alu.add, PAIRS,
                                   [xd_in], [xd_out])
                        xdbl = pB.tile([96, L], F32, tag="xdbl")
                        nc.sync.dma_start(xdbl[:], xd_out[:])

                        # ---------- Phase 4: dt proj + softplus ----------
                        dtw = pB.tile([DTR, DIH], BF16, tag="dtw")
                        nc.sync.dma_start(dtw[:], dtw_t[:])
                        dtraw = pB.tile([DTR, L], BF16, tag="dtraw")
                        nc.vector.tensor_copy(dtraw[:], xdbl[0:DTR, :])
                        # softplus(x) = ln(1 + exp(x)); batch all Exps then
                        # all Lns (in place through delta) so the ACT table
                        # only switches once between the exp and ln sets
                        for m in range(DT):
                            psd = [psum.tile([P, 512], F32, tag="mm",
                                             name="psd%d" % _h)
                                   for _h in range(2)]
                            for h in range(2):
                                nc.tensor.matmul(
                                    psd[h][:], dtw[:, m * P:(m + 1) * P],
                                    dtraw[:, h * 512:(h + 1) * 512],
                                    start=True, stop=True)
                                nc.scalar.activation(
                                    delta[:, m, h * 512:(h + 1) * 512],
                                    psd[h][:], Act.Exp, bias=dtb[:, m:m + 1])
                        for m in range(DT):
                            nc.scalar.activation(delta[:, m, :],
                                                 delta[:, m, :], Act.Ln,
                                                 bias=1.0)

                        # ------- Phase 5: B/C rows to DRAM (bf16) -------
                        bc_bf = pB.tile([2 * NST, L], BF16, tag="bc")
                        nc.vector.tensor_copy(bc_bf[:], xdbl[DTR:96, :])
                        nc.sync.dma_start(bc_dram[:], bc_bf[:])

                # prefetch out_proj weight (DMA overlaps the scan)
                nc.sync.dma_start(
                    outw[:], outw_t.rearrange("(o p) e -> p o e", p=P))

                # ---------- Phase 6: selective scan (n-major) ----------
                with tc.tile_pool(name="pC", bufs=1) as pC, \
                     tc.tile_pool(name="pC2", bufs=2) as pC2, \
                     tc.tile_pool(name="pC3", bufs=4) as pC3:
                    du = pC.tile([P, DT, L], BF16, tag="du")
                    for d in range(DT):
                        nc.vector.tensor_tensor(du[:, d, :], delta[:, d, :],
                                                u_bf[:, d, :], Alu.mult)
                    ya = pC.tile([P, DT, L], BF16, tag="ya")
                    for n in range(NST):
                        b_n = pC2.tile([P, L], BF16, tag="bn", name="b_n")
                        c_n = pC2.tile([P, L], BF16, tag="cn", name="c_n")
                        nc.sync.dma_start(
                            b_n[:], bc_dram[n:n + 1, :].to_broadcast((P, L)))
                        nc.sync.dma_start(
                            c_n[:],
                            bc_dram[NST + n:NST + n + 1, :].to_broadcast((P, L)))
                        for d in range(DT):
                            dA = pC3.tile([P, L], F32, tag="dA", name="dA")
                            nc.scalar.activation(dA[:], delta[:, d, :], Act.Exp,
                                                 scale=a_sb[:, d, n:n + 1])
                            dBu = pC3.tile([P, L], BF16, tag="dBu", name="dBu")
                            nc.vector.tensor_tensor(dBu[:], du[:, d, :], b_n[:],
                                                    Alu.mult)
                            h = pC3.tile([P, L], BF16, tag="h", name="h")
                            nc.vector.tensor_tensor_scan(h[:], dA[:], dBu[:],
                                                         0.0, Alu.mult,
                                                         Alu.add)
                            nc.vector.tensor_tensor(h[:], h[:], c_n[:], Alu.mult)
                            if n == 0:
                                nc.sync.dma_start(ya[:, d, :], h[:])
                            else:
                                nc.gpsimd.dma_start(ya[:, d, :], h[:],
                                                    accum_op=Alu.add)
                    # gate: y_g = (u*D + ya) * silu(z)
                    for d in range(DT):
                        nc.vector.scalar_tensor_tensor(
                            ya[:, d, :], u_bf[:, d, :], dpv[:, d:d + 1],
                            ya[:, d, :], Alu.mult, Alu.add)
                        nc.vector.tensor_tensor(y_g[:, d, :], ya[:, d, :],
                                                silz[:, d, :], Alu.mult)

            # ---------- Phase 7: out_proj + combine directions ----------
            with tc.tile_pool(name="pE", bufs=1) as pE:
                yp_in = dram.tile([4, P, KT * L // 4], F32)
                yp_out = dram.tile([4, P, KT * L // 4], F32)
                ag_out = dram.tile([4, 2, P, KT * L // 4], F32)
                ypv = yp_in[:].rearrange("s p (o t) -> s p o t", t=L)
                for m in range(KT):
                    pso = [psum.tile([P, 512], F32, tag="mm",
                                     name="pso%d" % _h) for _h in range(2)]
                    for d in range(DT):
                        for h in range(2):
                            nc.tensor.matmul(
                                pso[h][:], outw[:, d, m * P:(m + 1) * P],
                                y_g[:, d, h * 512:(h + 1) * 512],
                                start=(d == 0), stop=(d == DT - 1))
                    ypm = scratch.tile([P, L], F32, tag="big", name="ypm")
                    for h in range(2):
                        nc.scalar.copy(ypm[:, h * 512:(h + 1) * 512],
                                       pso[h][:])
                    nc.sync.dma_start(ypv[m // 2, :, m % 2, :], ypm[:])
                for hh in range(4):
                    collective("AllReduce", Alu.add, PAIRS,
                               [yp_in[hh]], [yp_out[hh]])
                    collective("AllGather", Alu.bypass, XDIR,
                               [yp_out[hh]],
                               [ag_out[hh, 0] if sim else ag_out[hh]])
                ag3 = ag_out[:].rearrange("h s p (o t) -> h s p o t", t=L)

                # FFN weights (DMA overlaps phase 7 compute)
                w1s = pE.tile([P, KT, DFQ], BF16, tag="w1s")
                nc.sync.dma_start(w1s[:],
                                  w1_t.rearrange("(o p) e -> p o e", p=P))
                w2s = pE.tile([P, DFQ // P, DM], BF16, tag="w2s")
                nc.sync.dma_start(w2s[:],
                                  w2_t.rearrange("(o p) e -> p o e", p=P))

                x2 = pE.tile([P, KT, L], F32, tag="x2")
                xrv = xT_res.rearrange("(o p) t -> p o t", p=P)
                for kt in range(KT):
                    s1 = scratch.tile([P, L], F32, tag="big", name="s1")
                    nc.sync.dma_start(s1[:], ag3[kt // 2, 1, :, kt % 2, :])
                    nc.sync.dma_start(x2[:, kt, :], xrv[:, kt, :])
                    nc.gpsimd.dma_start(x2[:, kt, :],
                                        ag3[kt // 2, 0, :, kt % 2, :],
                                        accum_op=Alu.add)
                    nc.vector.tensor_tensor(x2[:, kt, :], x2[:, kt, :],
                                            s1[:, ::-1], Alu.add)

                # store the residual part of the output now; the FFN
                # contribution is accumulated into outT at the end
                outv = outT.rearrange("(o p) t -> p o t", p=P)
                for kt in range(KT):
                    nc.sync.dma_start(outv[:, kt, :], x2[:, kt, :])

                # ---------- Phase 8: LN2 ----------
                mean2 = pE.tile([P, L], F32, tag="mean2")
                rstd2 = pE.tile([P, L], F32, tag="rstd2")
                _ln_stats(nc, pools, lambda kt: x2[:, kt, :], ones_col,
                          ones_bf, rstd2, mean2)
                x2n = pE.tile([P, KT, L], BF16, tag="x2n")
                _ln_apply(nc, pools, lambda kt: x2[:, kt, :], mean2, rstd2,
                          x2n)

                # ---------- Phase 9: FFN ----------
                h1 = pE.tile([P, DFQ // P, L], BF16, tag="h1")
                for m in range(DFQ // P):
                    psf = [psum.tile([P, 512], F32, tag="mm",
                                     name="psf%d" % _h) for _h in range(2)]
                    for kt in range(KT):
                        for h in range(2):
                            nc.tensor.matmul(
                                psf[h][:], w1s[:, kt, m * P:(m + 1) * P],
                                x2n[:, kt, h * 512:(h + 1) * 512],
                                start=(kt == 0), stop=(kt == KT - 1))
                    for h in range(2):
                        nc.scalar.activation(h1[:, m, h * 512:(h + 1) * 512],
                                             psf[h][:], Act.Gelu,
                                             bias=b1s[:, m:m + 1])
                ff_in = dram.tile([4, P, KT * L // 4], F32)
                ff_out = dram.tile([4, P, KT * L // 4], F32)
                ffv = ff_in[:].rearrange("s p (o t) -> s p o t", t=L)
                for m in range(KT):
                    psg = [psum.tile([P, 512], F32, tag="mm",
                                     name="psg%d" % _h) for _h in range(2)]
                    for kt in range(DFQ // P):
                        for h in range(2):
                            nc.tensor.matmul(
                                psg[h][:], w2s[:, kt, m * P:(m + 1) * P],
                                h1[:, kt, h * 512:(h + 1) * 512],
                                start=(kt == 0), stop=(kt == DFQ // P - 1))
                    ffm = scratch.tile([P, L], F32, tag="big", name="ffm")
                    for h in range(2):
                        nc.scalar.activation(ffm[:, h * 512:(h + 1) * 512],
                                             psg[h][:], Act.Identity,
                                             bias=b2s[:, m:m + 1])
                    nc.sync.dma_start(ffv[m // 2, :, m % 2, :], ffm[:])
                for hh in range(4):
                    collective("AllReduce", Alu.add, QUADS,
                               [ff_in[hh]], [ff_out[hh]])
                ffo = ff_out[:].rearrange("s p (o t) -> s p o t", t=L)
                for hh in range(4):
                    nc.gpsimd.dma_start(outv[:, hh * 2:(hh + 1) * 2, :],
                                        ffo[hh], accum_op=Alu.add)


_NC_CACHE = {}


def _get_nc(reps=1):
    if reps not in _NC_CACHE:
        _NC_CACHE[reps] = build_program(reps=reps)
    return _NC_CACHE[reps]


def _prep_core(inputs, dir_, b, half):
    hs = slice(half * DIH, (half + 1) * DIH)
    p = "f_" if dir_ == 0 else "b_"
    f32 = np.float32
    xT = np.ascontiguousarray(inputs["x"][b].T.astype(f32))
    m = {}
    m["xT_res"] = xT
    m["xT_ln"] = xT if dir_ == 0 else np.ascontiguousarray(xT[:, ::-1])
    W = inputs[p + "in_proj_w"]
    win = np.concatenate([W[hs], W[2 * DIH + half * DIH:
                                   2 * DIH + (half + 1) * DIH]], axis=0)
    g1 = inputs["norm_g"].astype(np.float64)
    b1n = inputs["norm_b"].astype(np.float64)
    m["win_t"] = np.ascontiguousarray(
        (win.astype(np.float64) * g1[None, :]).T).astype(_BF)
    m["win_b"] = (win.astype(np.float64) @ b1n).astype(f32)
    m["conv_w"] = np.ascontiguousarray(
        inputs[p + "conv_w"][hs, 0, :]).astype(f32)
    m["conv_b"] = inputs[p + "conv_b"][hs].astype(f32)
    m["a_mat"] = (-np.exp(inputs[p + "A_log"][hs])).astype(f32)
    m["xpw_t"] = np.ascontiguousarray(
        inputs[p + "x_proj_w"][:, hs].T).astype(_BF)
    m["dtw_t"] = np.ascontiguousarray(
        inputs[p + "dt_proj_w"][hs].T).astype(_BF)
    m["dt_b"] = inputs[p + "dt_proj_b"][hs].astype(f32)
    m["d_par"] = inputs[p + "D"][hs].astype(f32)
    m["outw_t"] = np.ascontiguousarray(
        0.5 * inputs[p + "out_proj_w"][:, hs].T).astype(_BF)
    q = 2 * dir_ + half
    qs = slice(q * DFQ, (q + 1) * DFQ)
    g2 = inputs["ffn_g"].astype(np.float64)
    b2n = inputs["ffn_b"].astype(np.float64)
    w1q = inputs["w1"][qs].astype(np.float64)
    m["w1_t"] = np.ascontiguousarray((w1q * g2[None, :]).T).astype(_BF)
    m["b1_q"] = (inputs["b1"][qs] + w1q @ b2n).astype(f32)
    m["w2_t"] = np.ascontiguousarray(inputs["w2"][:, qs].T).astype(_BF)
    m["b2_e"] = (inputs["b2"] if q == 0
                 else np.zeros_like(inputs["b2"])).astype(f32)
    return m


def make_in_maps(inputs):
    inputs = {k: np.asarray(v) for k, v in inputs.items()}
    maps = []
    for c in range(8):
        dir_, b, half = c // 4, (c % 4) // 2, c % 2
        maps.append(_prep_core(inputs, dir_, b, half))
    return maps


def kernel(**inputs):
    from concourse.bass_utils import run_bass_kernel_spmd
    nc = _get_nc()
    in_maps = make_in_maps(inputs)
    res = run_bass_kernel_spmd(nc, in_maps, core_ids=list(range(8)))
    out0 = res.results[0]["outT"]  # batch 0, [DM, L]
    out1 = res.results[2]["outT"]  # batch 1
    return np.stack([out0.T, out1.T]).astype(np.float32)

